# revision 2
# baseline (speedup 1.0000x reference)
"""GCN 2-layer encoder on 8 TRN2 NeuronCores (Bass/Tile).

Math (PyG GCNConv, symmetric normalization, self-loops, deg from dst):
    out1 = relu(Dh @ A @ Dh @ (x @ W1) + b1),  Dh = diag(deg^-1/2)
    out  = Dh @ A @ Dh @ (out1 @ W2) + b2

Since aggregation is linear, W is applied AFTER aggregation:
    out1[d] = relu(dinv[d] * (agg1[d] @ W1) + b1),  agg1[d] = sum_{s in N(d)+{d}} xs[s]
    xs[s]   = dinv[s] * x[s]                        (prescaled on HOST -> no phase 1!)
    out[d]  = dinv[d] * (agg2[d] @ W2) + b2,        agg2[d] = sum r3[s]
    r3[s]   = dinv[s] * out1[s]

Sharding: nodes -> 8 cores (balanced total degree). Each core aggregates its own
dst nodes via SWDGE dma_gather of 256B rows + multi-hot sigma matmuls in PSUM.
Self-loops never gathered: one identity matmul per tile adds the own-rows tile
(xs_own for layer 1; SBUF-resident r3 for layer 2).

Gather indices are int16 -> the row table is split in A (cores 0-3) / B halves.
Per (tile, side) edges are packed into chunk*lane cells under up to 3 "sigma
ranges" (chunk intervals sharing one sigma), which kills the old per-node
lane-capacity padding. 4 SWDGE queues + 3 staging groups in flight parallelize
descriptor generation (measured 3.3x vs serial issue).

Layer-2 rows (r3) are exchanged with one AllGather between the layers.
"""

import sys
import types

sys.path.insert(0, "/opt/trn_rl_repo")

import numpy as np

# Register the NTFF profile hook the container's antenv stub lacks, so
# BASS_TRACE=1 profiling works under axon (harmless otherwise).
if "antenv.axon_hooks" not in sys.modules:
    try:
        from trn_agent_boot.trn_boot import _ntff_profile_via_ctypes

        _hook = _ntff_profile_via_ctypes("/opt/axon/libaxon_pjrt.so")
    except Exception:
        _hook = None
    _m = types.ModuleType("antenv.axon_hooks")
    _m.get_axon_ntff_profile_hook = lambda: _hook
    sys.modules["antenv.axon_hooks"] = _m

N = 50000
E = 800000
IN_CH = 128
HID = 128
OUT_CH = 64
NCORES = 8
P = 128
GSZ = 4  # tiles per gather/stage group
CALL_CAP = 8  # max chunks (x128 idxs) per dma_gather call; larger crashes HW
SWDGE_QUEUES = 4
STG_BUFS = 3  # staging groups in flight

_CACHE = {}
LAST_RESULTS = None


# ----------------------------------------------------------------------------
# Host-side planning
# ----------------------------------------------------------------------------
def _range_split(C):
    """Split C chunks into <=3 sigma-range lengths."""
    l1 = (C + 1) // 2
    l2 = (C - l1 + 1) // 2
    l3 = C - l1 - l2
    return [l for l in (l1, l2, l3) if l > 0]


def _pack_side(counts, C):
    """Pack nodes with cell-counts `counts` (list of (col, c)) into 128 lanes
    x C chunks under sigma ranges _range_split(C).

    A lane column is split into one segment per range; each segment belongs to
    one node. Full lanes (all ranges, same node) are allocated from lane 0 up;
    partial segments from lane 127 down, per range independently.

    Returns None if infeasible, else (range_lengths, [(col, [(rho, lane,
    n_used), ...]), ...])."""
    L = _range_split(C)
    R = len(L)
    nf = 0  # full lanes used (lanes 0..nf-1)
    u = [0] * R  # partial segments used per range (lanes 127-u[rho]+1..127)
    out = []
    for col, c in sorted(counts, key=lambda t: -t[1]):
        segs = []
        f = c // C
        rem = c - f * C
        if nf + f > 128 or any(nf + f + uu > 128 for uu in u):
            return None
        for i in range(f):
            lane = nf + i
            for rho in range(R):
                segs.append((rho, lane, L[rho]))
        nf += f
        # remainder: greedy largest L[rho] <= rem, else smallest available
        while rem > 0:
            best = -1
            for rho in range(R):
                if nf + u[rho] >= 128:
                    continue
                if L[rho] <= rem and (best < 0 or L[rho] > L[best]):
                    best = rho
            if best < 0:
                cand = [rho for rho in range(R) if nf + u[rho] < 128]
                if not cand:
                    return None
                best = min(cand, key=lambda rho: L[rho])
            lane = 127 - u[best]
            used = min(rem, L[best])
            segs.append((best, lane, used))
            u[best] += 1
            rem -= used
        out.append((col, segs))
    return L, out


def _min_C(counts):
    tot = sum(c for _, c in counts)
    C = max(1, -(-tot // 128))
    while True:
        if _pack_side(counts, C) is not None:
            return C
        C += 1


def _plan(edge_index):
    src = np.asarray(edge_index[0], dtype=np.int64)
    dst = np.asarray(edge_index[1], dtype=np.int64)
    deg = np.bincount(dst, minlength=N) + 1  # + self-loop
    dinv = (1.0 / np.sqrt(deg.astype(np.float64))).astype(np.float32)

    # --- node -> core (snake over degree-sorted nodes: balances sum(deg)) ---
    order = np.argsort(-deg, kind="stable")
    snake = np.tile(
        np.concatenate([np.arange(NCORES), np.arange(NCORES - 1, -1, -1)]),
        N // (2 * NCORES) + 1,
    )[:N]
    core_of = np.empty(N, dtype=np.int64)
    core_of[order] = snake

    T_REAL = -(-(N // NCORES) // P)  # 49
    T = T_REAL + 1  # +1 empty tile = guaranteed zero rows for PAD index
    SLOTS = T * P
    HALF = (NCORES // 2) * SLOTS
    assert HALF <= 32768, (T, SLOTS)

    # --- per-dst A/B in-edge counts (A = src on cores 0-3), no self-loops ---
    isA = core_of[src] < (NCORES // 2)
    a_cnt = np.bincount(dst[isA], minlength=N)
    b_cnt = np.bincount(dst[~isA], minlength=N)

    # --- nodes -> tiles (LPT greedy on a+b, <=128 per tile) -----------------
    core_tiles = []  # [core] -> list of T_REAL node-id lists
    for c in range(NCORES):
        nodes = np.where(core_of == c)[0]
        w = a_cnt[nodes] + b_cnt[nodes]
        o2 = np.argsort(-w, kind="stable")
        loads = np.zeros(T_REAL, dtype=np.int64)
        fills = np.zeros(T_REAL, dtype=np.int64)
        tiles = [[] for _ in range(T_REAL)]
        for i in o2:
            open_ = np.where(fills < P)[0]
            t = open_[np.argmin(loads[open_])]
            tiles[t].append(nodes[i])
            loads[t] += w[i]
            fills[t] += 1
        o3 = np.argsort(-loads, kind="stable")  # sort tiles desc by load
        core_tiles.append([tiles[t] for t in o3])

    # --- per (core, tile, side) minimal chunk counts ------------------------
    minC = np.zeros((NCORES, T_REAL, 2), dtype=np.int64)
    counts_all = {}
    for c in range(NCORES):
        for k in range(T_REAL):
            nds = core_tiles[c][k]
            ca = [(j, int(a_cnt[nd])) for j, nd in enumerate(nds) if a_cnt[nd] > 0]
            cb = [(j, int(b_cnt[nd])) for j, nd in enumerate(nds) if b_cnt[nd] > 0]
            counts_all[(c, k)] = (ca, cb)
            minC[c, k, 0] = _min_C(ca) if ca else 0
            minC[c, k, 1] = _min_C(cb) if cb else 0
    CA = minC[:, :, 0].max(axis=0)  # shared per-tile chunk counts (SPMD)
    CB = minC[:, :, 1].max(axis=0)

    # --- slot/position assignment ------------------------------------------
    slot_of = np.full(N, -1, dtype=np.int64)
    for c in range(NCORES):
        for k in range(T_REAL):
            for j, nd in enumerate(core_tiles[c][k]):
                slot_of[nd] = k * P + j
    pos_of = core_of * SLOTS + slot_of

    # --- edge lists grouped by (dst, side), sources as table positions ------
    sideval = (~isA).astype(np.int64)
    eorder = np.argsort(dst * 2 + sideval, kind="stable")
    src_pos_sorted = pos_of[src[eorder]]
    estart = np.zeros(N + 1, dtype=np.int64)
    np.cumsum(a_cnt + b_cnt, out=estart[1:])

    # --- groups -------------------------------------------------------------
    G = -(-T_REAL // GSZ)
    groups = [list(range(g * GSZ, min((g + 1) * GSZ, T_REAL))) for g in range(G)]
    PAD = SLOTS - 1  # empty-tile row of core 0 (A half) / core 4 (B half)

    # --- sigma layout: per tile [A ranges..., B ranges...], stacked ---------
    tot_chunks = int(CA.sum() + CB.sum())
    rangesA = [_range_split(int(C)) if C else [] for C in CA]
    rangesB = [_range_split(int(C)) if C else [] for C in CB]
    nsig_tile = [len(rangesA[k]) + len(rangesB[k]) for k in range(T_REAL)]
    sig_off = np.zeros(T_REAL + 1, dtype=np.int64)
    np.cumsum(nsig_tile, out=sig_off[1:])
    NSIG = int(sig_off[-1])

    idx_cores = []
    sigma_cores = []
    dinv_own_cores = []
    xs_perm = np.full((NCORES, SLOTS), N, dtype=np.int64)  # node per slot (N=zero)
    for c in range(NCORES):
        blocksA = [np.full((int(CA[k]), P), PAD, np.int64) for k in range(T_REAL)]
        blocksB = [np.full((int(CB[k]), P), PAD, np.int64) for k in range(T_REAL)]
        sig = np.zeros((NSIG, P, P), dtype=np.float16)
        dvo = np.zeros((P, T), dtype=np.float32)
        for k in range(T_REAL):
            nds = core_tiles[c][k]
            for j, nd in enumerate(nds):
                dvo[j, k] = dinv[nd]
                xs_perm[c, k * P + j] = nd
            ca, cb = counts_all[(c, k)]
            for counts, C, rngs, blocks, soff in (
                (ca, int(CA[k]), rangesA[k], blocksA, 0),
                (cb, int(CB[k]), rangesB[k], blocksB, len(rangesA[k])),
            ):
                if not counts:
                    continue
                packed = _pack_side(counts, C)
                assert packed is not None, (c, k, C)
                L, placements = packed
                roff = np.zeros(len(L) + 1, dtype=np.int64)
                np.cumsum(L, out=roff[1:])
                for col, segs in placements:
                    nd = nds[col]
                    s0 = int(estart[nd])
                    na = int(a_cnt[nd])
                    if soff == 0:  # A side
                        elist = src_pos_sorted[s0 : s0 + na]
                    else:
                        elist = src_pos_sorted[s0 + na : int(estart[nd + 1])] - HALF
                    epos = 0
                    for rho, lane, used in segs:
                        sig[sig_off[k] + soff + rho, lane, col] = 1.0
                        blocks[k][roff[rho] : roff[rho] + used, lane] = elist[
                            epos : epos + used
                        ]
                        epos += used
                    assert epos == len(elist), (c, k, col)
        # flatten idx in group order: per group: A chunks (tile-major), then B
        flat = []
        for g in groups:
            for k in g:
                flat.append(blocksA[k].reshape(-1))
            for k in g:
                flat.append(blocksB[k].reshape(-1))
        flat = np.concatenate(flat)
        assert flat.size == tot_chunks * P
        assert flat.min() >= 0 and flat.max() < HALF
        wrapped = flat.astype(np.int16).reshape(-1, 16).T.copy()
        idx_cores.append(np.tile(wrapped, (8, 1)))
        sigma_cores.append(sig)
        dinv_own_cores.append(dvo)

    return dict(
        T=T,
        T_REAL=T_REAL,
        SLOTS=SLOTS,
        CA=CA,
        CB=CB,
        rangesA=rangesA,
        rangesB=rangesB,
        sig_off=sig_off,
        NSIG=NSIG,
        groups=groups,
        tot_chunks=tot_chunks,
        core_of=core_of,
        slot_of=slot_of,
        pos_of=pos_of,
        dinv=dinv,
        idx_cores=idx_cores,
        sigma_cores=sigma_cores,
        dinv_own_cores=dinv_own_cores,
        xs_perm=xs_perm,
    )


# ----------------------------------------------------------------------------
# Device kernel
# ----------------------------------------------------------------------------
def _build(plan):
    import concourse.bass as bass
    import concourse.mybir as mybir
    import concourse.tile as tile
    from concourse import bacc

    f16 = mybir.dt.float16
    f32 = mybir.dt.float32
    i16 = mybir.dt.int16

    T = plan["T"]
    T_REAL = plan["T_REAL"]
    SLOTS = plan["SLOTS"]
    CA, CB = plan["CA"], plan["CB"]
    rangesA, rangesB = plan["rangesA"], plan["rangesB"]
    sig_off = plan["sig_off"]
    NSIG = plan["NSIG"]
    groups = plan["groups"]
    tot_chunks = plan["tot_chunks"]
    ROWS = NCORES * SLOTS
    HALFROWS = ROWS // 2
    max_ca = max(int(sum(CA[k] for k in g)) for g in groups)
    max_cb = max(int(sum(CB[k] for k in g)) for g in groups)
    max_sig = max(int(sig_off[g[-1] + 1] - sig_off[g[0]]) for g in groups)

    nc = bacc.Bacc(
        "TRN2",
        target_bir_lowering=False,
        num_devices=NCORES,
        num_swdge_queues=SWDGE_QUEUES,
    )
    qn = [0]

    def _next_q():
        qn[0] = (qn[0] + 1) % SWDGE_QUEUES
        return qn[0]

    xs_in = nc.dram_tensor("xs", [ROWS, IN_CH], f16, kind="ExternalInput")
    xso_in = nc.dram_tensor("xs_own", [SLOTS, IN_CH], f16, kind="ExternalInput")
    w1_in = nc.dram_tensor("W1", [IN_CH, HID], f16, kind="ExternalInput")
    w2_in = nc.dram_tensor("W2", [HID, OUT_CH], f16, kind="ExternalInput")
    b1_in = nc.dram_tensor("b1bc", [P, HID], f32, kind="ExternalInput")
    b2_in = nc.dram_tensor("b2bc", [P, OUT_CH], f32, kind="ExternalInput")
    id_in = nc.dram_tensor("ident", [P, P], f16, kind="ExternalInput")
    sig_in = nc.dram_tensor("sigma", [NSIG, P, P], f16, kind="ExternalInput")
    do_in = nc.dram_tensor("dinv_own", [P, T], f32, kind="ExternalInput")
    idx_in = nc.dram_tensor("idx", [P, tot_chunks * 8], i16, kind="ExternalInput")
    out_ext = nc.dram_tensor("out", [SLOTS, OUT_CH], f32, kind="ExternalOutput")

    with tile.TileContext(nc) as tc:
        with (
            tc.tile_pool(name="const", bufs=1) as cpool,
            tc.tile_pool(name="sig", bufs=STG_BUFS) as sigpool,
            tc.tile_pool(name="stg", bufs=STG_BUFS) as stgpool,
            tc.tile_pool(name="own", bufs=STG_BUFS) as ownpool,
            tc.tile_pool(name="drain", bufs=3) as dpool,
            tc.tile_pool(name="wb", bufs=2) as wbpool,
            tc.tile_pool(name="psa", bufs=3, space="PSUM") as ps_agg,
            tc.tile_pool(name="pst", bufs=2, space="PSUM") as ps_tr,
            tc.tile_pool(name="psm", bufs=2, space="PSUM") as ps_mm,
            tc.tile_pool(name="dram", bufs=1, space="DRAM") as dram,
        ):
            # ---- constants into SBUF ----
            w1_sb = cpool.tile([IN_CH, HID], f16)
            nc.sync.dma_start(out=w1_sb[:], in_=w1_in[:])
            w2_sb = cpool.tile([HID, OUT_CH], f16)
            nc.sync.dma_start(out=w2_sb[:], in_=w2_in[:])
            b1_sb = cpool.tile([P, HID], f32)
            nc.sync.dma_start(out=b1_sb[:], in_=b1_in[:])
            b2_sb = cpool.tile([P, OUT_CH], f32)
            nc.sync.dma_start(out=b2_sb[:], in_=b2_in[:])
            id_sb = cpool.tile([P, P], f16)
            nc.sync.dma_start(out=id_sb[:], in_=id_in[:])
            do_sb = cpool.tile([P, T], f32)
            nc.sync.dma_start(out=do_sb[:], in_=do_in[:])
            idx_sb = cpool.tile([P, tot_chunks * 8], i16)
            nc.sync.dma_start(out=idx_sb[:], in_=idx_in[:])
            r3_sb = cpool.tile([P, T_REAL * P], f16)  # layer-2 own rows

            shard2 = dram.tile([SLOTS, HID], f16)
            table2 = dram.tile([ROWS, HID], f16, addr_space="Shared")

            # zero the spare tile of shard2 (PAD rows of table2)
            zt = cpool.tile([P, HID], f16)
            nc.vector.memset(zt[:], 0.0)
            nc.sync.dma_start(out=shard2[T_REAL * P : SLOTS, :], in_=zt[:])

            def drain(layer, k, gi, aps, wb):
                dv = do_sb[:, k : k + 1]
                # agg (PSUM f32) -> f16, transpose, then apply W
                s16 = dpool.tile([P, P], f16, tag="s16")
                nc.scalar.activation(s16[:], aps[:], mybir.ActivationFunctionType.Copy)
                psT = ps_tr.tile([P, P], f16, tag="tr")
                nc.tensor.transpose(psT[:], s16[:], id_sb[:])
                rT = dpool.tile([P, P], f16, tag="rT")
                nc.vector.tensor_copy(rT[:], psT[:])
                if layer == 0:
                    ps2 = ps_mm.tile([P, HID], f32, tag="mm")
                    nc.tensor.matmul(
                        ps2[:], lhsT=rT[:], rhs=w1_sb[:], start=True, stop=True
                    )
                    r1 = dpool.tile([P, HID], f32, tag="r1")
                    nc.scalar.activation(
                        r1[:], ps2[:], mybir.ActivationFunctionType.Copy, scale=dv
                    )
                    nc.vector.tensor_add(r1[:], r1[:], b1_sb[:])
                    # r3 = relu(r1) * dinv -> f16, into resident r3_sb AND wb
                    r3s = r3_sb[:, k * P : (k + 1) * P]
                    nc.vector.tensor_scalar(
                        r3s, r1[:], 0.0, dv, mybir.AluOpType.max, mybir.AluOpType.mult
                    )
                    nc.vector.tensor_copy(wb[:, gi, :], r3s)
                else:
                    ps2 = ps_mm.tile([P, OUT_CH], f32, tag="mm")
                    nc.tensor.matmul(
                        ps2[:], lhsT=rT[:], rhs=w2_sb[:], start=True, stop=True
                    )
                    o1 = dpool.tile([P, OUT_CH], f32, tag="o1")
                    nc.scalar.activation(
                        o1[:], ps2[:], mybir.ActivationFunctionType.Copy, scale=dv
                    )
                    nc.vector.tensor_add(wb[:, gi, :], o1[:], b2_sb[:])

            def aggregate(layer):
                tab = xs_in if layer == 0 else table2
                coff = 0
                for g in groups:
                    ca_g = int(sum(int(CA[k]) for k in g))
                    cb_g = int(sum(int(CB[k]) for k in g))
                    stA = stB = None
                    if ca_g:
                        stA = stgpool.tile([P, max_ca, P], f16, tag="stgA")
                        for s_ in range(0, ca_g, CALL_CAP):
                            n_ = min(CALL_CAP, ca_g - s_)
                            nc.gpsimd.dma_gather(
                                stA[:, s_ : s_ + n_, :],
                                tab[0:HALFROWS, :],
                                idx_sb[:, (coff + s_) * 8 : (coff + s_ + n_) * 8],
                                n_ * P,
                                n_ * P,
                                P,
                                queue_num=_next_q(),
                            )
                    if cb_g:
                        stB = stgpool.tile([P, max_cb, P], f16, tag="stgB")
                        for s_ in range(0, cb_g, CALL_CAP):
                            n_ = min(CALL_CAP, cb_g - s_)
                            nc.gpsimd.dma_gather(
                                stB[:, s_ : s_ + n_, :],
                                tab[HALFROWS:ROWS, :],
                                idx_sb[
                                    :,
                                    (coff + ca_g + s_) * 8 : (coff + ca_g + s_ + n_)
                                    * 8,
                                ],
                                n_ * P,
                                n_ * P,
                                P,
                                queue_num=_next_q(),
                            )
                    # sigma slab + own rows for this group (batched dmas)
                    s0, s1 = int(sig_off[g[0]]), int(sig_off[g[-1] + 1])
                    sg = sigpool.tile([P, max_sig, P], f16, tag="sig")
                    if s1 > s0:
                        nc.sync.dma_start(
                            out=sg[:, 0 : s1 - s0, :],
                            in_=sig_in[s0:s1].rearrange("s p q -> p s q"),
                        )
                    if layer == 0:
                        own = ownpool.tile([P, GSZ, IN_CH], f16, tag="own")
                        nc.sync.dma_start(
                            out=own[:, 0 : len(g), :],
                            in_=xso_in[g[0] * P : (g[-1] + 1) * P, :].rearrange(
                                "(t p) f -> p t f", p=P
                            ),
                        )
                    wb = wbpool.tile(
                        [P, GSZ, HID if layer == 0 else OUT_CH],
                        f16 if layer == 0 else f32,
                        tag="wb0" if layer == 0 else "wb1",
                    )
                    a_off = 0
                    b_off = 0
                    for gi, k in enumerate(g):
                        aps = ps_agg.tile([P, P], f32, tag="agg")
                        # self-loop rows via identity matmul (starts PSUM)
                        own_rhs = (
                            own[:, gi, :]
                            if layer == 0
                            else r3_sb[:, k * P : (k + 1) * P]
                        )
                        nchunks = int(CA[k]) + int(CB[k])
                        nc.tensor.matmul(
                            aps[:],
                            lhsT=id_sb[:],
                            rhs=own_rhs,
                            start=True,
                            stop=(nchunks == 0),
                        )
                        done = 0
                        soff = int(sig_off[k]) - s0
                        for Cn, rngs, st, off0 in (
                            (int(CA[k]), rangesA[k], stA, a_off),
                            (int(CB[k]), rangesB[k], stB, b_off),
                        ):
                            ci = 0
                            for L in rngs:
                                sl = sg[:, soff, :]
                                for _ in range(L):
                                    nc.tensor.matmul(
                                        aps[:],
                                        lhsT=sl,
                                        rhs=st[:, off0 + ci, :],
                                        start=False,
                                        stop=(done == nchunks - 1),
                                    )
                                    ci += 1
                                    done += 1
                                soff += 1
                        a_off += int(CA[k])
                        b_off += int(CB[k])
                        drain(layer, k, gi, aps, wb)
                    # group writeback (1 dma)
                    dst = shard2 if layer == 0 else out_ext
                    nc.sync.dma_start(
                        out=dst[g[0] * P : (g[-1] + 1) * P, :].rearrange(
                            "(t p) f -> p t f", p=P
                        ),
                        in_=wb[:, 0 : len(g), :],
                    )
                    coff += ca_g + cb_g

            aggregate(0)

            nc.gpsimd.collective_compute(
                "AllGather",
                mybir.AluOpType.bypass,
                replica_groups=[list(range(NCORES))],
                ins=[shard2.opt()],
                outs=[table2.opt()],
            )

            aggregate(1)

    nc.compile()  # bacc passes: library loads, register allocation, DCE
    import concourse.mybir as mybir2

    _split_sync_waits(nc, mybir2, max_waits=1)
    return nc


def _split_sync_waits(nc, mybir, max_waits=1):
    """This walrus build rejects instructions with more than `max_waits` sync
    waits; hoist excess waits onto injected same-engine InstNoOps."""
    n_split = 0
    for fn in nc.m.functions:
        for bb in fn.blocks:
            out = []
            changed = False
            for ins in bb.instructions:
                si = ins.sync_info
                if si is not None and si.on_wait and len(si.on_wait) > max_waits:
                    waits = list(si.on_wait)
                    excess = waits[:-max_waits]
                    for i in range(0, len(excess), max_waits):
                        nop = mybir.InstNoOp(
                            name=nc.get_next_instruction_name(),
                            sync_info=mybir.SyncInfo(
                                on_wait=excess[i : i + max_waits], on_update=[]
                            ),
                            bass_nofuse=True,
                            engine=ins.engine,
                        )
                        out.append(nop)
                        n_split += 1
                    si.on_wait = waits[-max_waits:]
                    ins.sync_info = si
                    changed = True
                out.append(ins)
            if changed:
                bb.instructions = out
    return n_split


# ----------------------------------------------------------------------------
# Entry point
# ----------------------------------------------------------------------------
def kernel(x, edge_index, W1, b1, W2, b2):
    global LAST_RESULTS
    from concourse.bass_utils import run_bass_kernel_spmd

    x = np.asarray(x)
    W1a = np.asarray(W1)
    b1a = np.asarray(b1)
    W2a = np.asarray(W2)
    b2a = np.asarray(b2)

    key = hash(np.asarray(edge_index)[:, :: E // 997].tobytes())
    if key not in _CACHE:
        plan = _plan(edge_index)
        nc = _build(plan)
        _CACHE[key] = (plan, nc)
    plan, nc = _CACHE[key]

    SLOTS = plan["SLOTS"]

    # xs table: dinv-prescaled x rows in slot layout, f16; zeros elsewhere
    xs_node = np.concatenate(
        [plan["dinv"][:, None] * x, np.zeros((1, IN_CH), np.float32)], axis=0
    ).astype(np.float16)
    xs = xs_node[plan["xs_perm"].reshape(-1)]  # [ROWS, IN_CH]

    in_common = {
        "xs": xs,
        "W1": W1a.astype(np.float16),
        "W2": W2a.astype(np.float16),
        "b1bc": np.broadcast_to(b1a.astype(np.float32), (P, HID)).copy(),
        "b2bc": np.broadcast_to(b2a.astype(np.float32), (P, OUT_CH)).copy(),
        "ident": np.eye(P, dtype=np.float16),
    }
    in_maps = []
    for c in range(NCORES):
        m = dict(in_common)
        m["xs_own"] = np.ascontiguousarray(xs[c * SLOTS : (c + 1) * SLOTS])
        m["sigma"] = plan["sigma_cores"][c]
        m["dinv_own"] = plan["dinv_own_cores"][c]
        m["idx"] = plan["idx_cores"][c]
        in_maps.append(m)

    res = run_bass_kernel_spmd(nc, in_maps, core_ids=list(range(NCORES)))
    LAST_RESULTS = res

    out = np.empty((N, OUT_CH), dtype=np.float32)
    core_of = plan["core_of"]
    slot_of = plan["slot_of"]
    for c in range(NCORES):
        sel = core_of == c
        out[sel] = res.results[c]["out"][slot_of[sel]]
    return out


# revision 11
# speedup vs baseline: 1.0028x; 1.0028x over previous
"""GCN 2-layer encoder on 8 TRN2 NeuronCores (Bass/Tile).

Math (PyG GCNConv, symmetric normalization, self-loops, deg from dst):
    out1 = relu(Dh @ A @ Dh @ (x @ W1) + b1),  Dh = diag(deg^-1/2)
    out  = Dh @ A @ Dh @ (out1 @ W2) + b2

Since aggregation is linear, W is applied AFTER aggregation:
    out1[d] = relu(dinv[d] * (agg1[d] @ W1) + b1),  agg1[d] = sum_{s in N(d)+{d}} xs[s]
    xs[s]   = dinv[s] * x[s]                        (prescaled on HOST -> no phase 1!)
    out[d]  = dinv[d] * (agg2[d] @ W2) + b2,        agg2[d] = sum r3[s]
    r3[s]   = dinv[s] * out1[s]

Sharding: nodes -> 8 cores (balanced total degree). Each core aggregates its own
dst nodes via SWDGE dma_gather of 256B rows + multi-hot sigma matmuls in PSUM.
Self-loops never gathered: one identity matmul per tile adds the own-rows tile
(xs_own for layer 1; SBUF-resident r3 for layer 2).

Gather indices are int16 -> the row table is split in A (cores 0-3) / B halves.
Per (tile, side) edges are packed into chunk*lane cells under up to 3 "sigma
ranges" (chunk intervals sharing one sigma), which kills the old per-node
lane-capacity padding. 4 SWDGE queues + 3 staging groups in flight parallelize
descriptor generation (measured 3.3x vs serial issue).

Layer-2 rows (r3) are exchanged with one AllGather between the layers.
"""

import sys
import types

sys.path.insert(0, "/opt/trn_rl_repo")

import numpy as np

# Register the NTFF profile hook the container's antenv stub lacks, so
# BASS_TRACE=1 profiling works under axon (harmless otherwise).
if "antenv.axon_hooks" not in sys.modules:
    try:
        from trn_agent_boot.trn_boot import _ntff_profile_via_ctypes

        _hook = _ntff_profile_via_ctypes("/opt/axon/libaxon_pjrt.so")
    except Exception:
        _hook = None
    _m = types.ModuleType("antenv.axon_hooks")
    _m.get_axon_ntff_profile_hook = lambda: _hook
    sys.modules["antenv.axon_hooks"] = _m

N = 50000
E = 800000
IN_CH = 128
HID = 128
OUT_CH = 64
NCORES = 8
P = 128
GSZ = 4  # tiles per gather/stage group
CALL_CAP = 8  # max chunks (x128 idxs) per dma_gather call; larger crashes HW
SWDGE_QUEUES = 4
STG_BUFS = 4  # staging groups in flight
AG_LAG = 2  # groups between a shard2 write and its partial AllGather dispatch

_CACHE = {}
LAST_RESULTS = None


# ----------------------------------------------------------------------------
# Host-side planning
# ----------------------------------------------------------------------------
def _range_split(C):
    """Split C chunks into <=3 sigma-range lengths."""
    l1 = (C + 1) // 2
    l2 = (C - l1 + 1) // 2
    l3 = C - l1 - l2
    return [l for l in (l1, l2, l3) if l > 0]


def _pack_side(counts, C):
    """Pack nodes with cell-counts `counts` (list of (col, c)) into 128 lanes
    x C chunks under sigma ranges _range_split(C).

    A lane column is split into one segment per range; each segment belongs to
    one node. Full lanes (all ranges, same node) are allocated from lane 0 up;
    partial segments from lane 127 down, per range independently.

    Returns None if infeasible, else (range_lengths, [(col, [(rho, lane,
    n_used), ...]), ...])."""
    L = _range_split(C)
    R = len(L)
    nf = 0  # full lanes used (lanes 0..nf-1)
    u = [0] * R  # partial segments used per range (lanes 127-u[rho]+1..127)
    out = []
    for col, c in sorted(counts, key=lambda t: -t[1]):
        segs = []
        f = c // C
        rem = c - f * C
        if nf + f > 128 or any(nf + f + uu > 128 for uu in u):
            return None
        for i in range(f):
            lane = nf + i
            for rho in range(R):
                segs.append((rho, lane, L[rho]))
        nf += f
        # remainder: greedy largest L[rho] <= rem, else smallest available
        while rem > 0:
            best = -1
            for rho in range(R):
                if nf + u[rho] >= 128:
                    continue
                if L[rho] <= rem and (best < 0 or L[rho] > L[best]):
                    best = rho
            if best < 0:
                cand = [rho for rho in range(R) if nf + u[rho] < 128]
                if not cand:
                    return None
                best = min(cand, key=lambda rho: L[rho])
            lane = 127 - u[best]
            used = min(rem, L[best])
            segs.append((best, lane, used))
            u[best] += 1
            rem -= used
        out.append((col, segs))
    return L, out


def _min_C(counts):
    tot = sum(c for _, c in counts)
    C = max(1, -(-tot // 128))
    while True:
        if _pack_side(counts, C) is not None:
            return C
        C += 1


def _plan(edge_index):
    src = np.asarray(edge_index[0], dtype=np.int64)
    dst = np.asarray(edge_index[1], dtype=np.int64)
    deg = np.bincount(dst, minlength=N) + 1  # + self-loop
    dinv = (1.0 / np.sqrt(deg.astype(np.float64))).astype(np.float32)

    # --- node -> core (snake over degree-sorted nodes: balances sum(deg)) ---
    order = np.argsort(-deg, kind="stable")
    snake = np.tile(
        np.concatenate([np.arange(NCORES), np.arange(NCORES - 1, -1, -1)]),
        N // (2 * NCORES) + 1,
    )[:N]
    core_of = np.empty(N, dtype=np.int64)
    core_of[order] = snake

    T_REAL = -(-(N // NCORES) // P)  # 49
    T = T_REAL + 1  # +1 empty tile = guaranteed zero rows for PAD index
    SLOTS = T * P
    HALF = (NCORES // 2) * SLOTS
    assert HALF <= 32768, (T, SLOTS)

    # --- per-dst A/B in-edge counts (A = src on cores 0-3), no self-loops ---
    isA = core_of[src] < (NCORES // 2)
    a_cnt = np.bincount(dst[isA], minlength=N)
    b_cnt = np.bincount(dst[~isA], minlength=N)

    # --- nodes -> tiles (LPT greedy on a+b, <=128 per tile) -----------------
    core_tiles = []  # [core] -> list of T_REAL node-id lists
    for c in range(NCORES):
        nodes = np.where(core_of == c)[0]
        w = a_cnt[nodes] + b_cnt[nodes]
        o2 = np.argsort(-w, kind="stable")
        loads = np.zeros(T_REAL, dtype=np.int64)
        fills = np.zeros(T_REAL, dtype=np.int64)
        tiles = [[] for _ in range(T_REAL)]
        for i in o2:
            open_ = np.where(fills < P)[0]
            t = open_[np.argmin(loads[open_])]
            tiles[t].append(nodes[i])
            loads[t] += w[i]
            fills[t] += 1
        o3 = np.argsort(-loads, kind="stable")  # sort tiles desc by load
        core_tiles.append([tiles[t] for t in o3])

    # --- per (core, tile, side) minimal chunk counts ------------------------
    minC = np.zeros((NCORES, T_REAL, 2), dtype=np.int64)
    counts_all = {}
    for c in range(NCORES):
        for k in range(T_REAL):
            nds = core_tiles[c][k]
            ca = [(j, int(a_cnt[nd])) for j, nd in enumerate(nds) if a_cnt[nd] > 0]
            cb = [(j, int(b_cnt[nd])) for j, nd in enumerate(nds) if b_cnt[nd] > 0]
            counts_all[(c, k)] = (ca, cb)
            minC[c, k, 0] = _min_C(ca) if ca else 0
            minC[c, k, 1] = _min_C(cb) if cb else 0
    CA = minC[:, :, 0].max(axis=0)  # shared per-tile chunk counts (SPMD)
    CB = minC[:, :, 1].max(axis=0)

    # --- slot/position assignment ------------------------------------------
    slot_of = np.full(N, -1, dtype=np.int64)
    for c in range(NCORES):
        for k in range(T_REAL):
            for j, nd in enumerate(core_tiles[c][k]):
                slot_of[nd] = k * P + j
    pos_of = core_of * SLOTS + slot_of

    # --- edge lists grouped by (dst, side), sources as table positions ------
    sideval = (~isA).astype(np.int64)
    eorder = np.argsort(dst * 2 + sideval, kind="stable")
    src_pos_sorted = pos_of[src[eorder]]
    estart = np.zeros(N + 1, dtype=np.int64)
    np.cumsum(a_cnt + b_cnt, out=estart[1:])

    # --- groups -------------------------------------------------------------
    G = -(-T_REAL // GSZ)
    groups = [list(range(g * GSZ, min((g + 1) * GSZ, T_REAL))) for g in range(G)]
    PAD = SLOTS - 1  # empty-tile row of core 0 (A half) / core 4 (B half)

    # --- sigma layout: per tile [A ranges..., B ranges...], stacked ---------
    tot_chunks = int(CA.sum() + CB.sum())
    rangesA = [_range_split(int(C)) if C else [] for C in CA]
    rangesB = [_range_split(int(C)) if C else [] for C in CB]
    nsig_tile = [len(rangesA[k]) + len(rangesB[k]) for k in range(T_REAL)]
    sig_off = np.zeros(T_REAL + 1, dtype=np.int64)
    np.cumsum(nsig_tile, out=sig_off[1:])
    NSIG = int(sig_off[-1])

    idx_cores = []
    sigma_cores = []
    dinv_own_cores = []
    xs_perm = np.full((NCORES, SLOTS), N, dtype=np.int64)  # node per slot (N=zero)
    for c in range(NCORES):
        blocksA = [np.full((int(CA[k]), P), PAD, np.int64) for k in range(T_REAL)]
        blocksB = [np.full((int(CB[k]), P), PAD, np.int64) for k in range(T_REAL)]
        sig = np.zeros((NSIG, P, P), dtype=np.float16)
        dvo = np.zeros((P, T), dtype=np.float32)
        for k in range(T_REAL):
            nds = core_tiles[c][k]
            for j, nd in enumerate(nds):
                dvo[j, k] = dinv[nd]
                xs_perm[c, k * P + j] = nd
            ca, cb = counts_all[(c, k)]
            for counts, C, rngs, blocks, soff in (
                (ca, int(CA[k]), rangesA[k], blocksA, 0),
                (cb, int(CB[k]), rangesB[k], blocksB, len(rangesA[k])),
            ):
                if not counts:
                    continue
                packed = _pack_side(counts, C)
                assert packed is not None, (c, k, C)
                L, placements = packed
                roff = np.zeros(len(L) + 1, dtype=np.int64)
                np.cumsum(L, out=roff[1:])
                for col, segs in placements:
                    nd = nds[col]
                    s0 = int(estart[nd])
                    na = int(a_cnt[nd])
                    if soff == 0:  # A side
                        elist = src_pos_sorted[s0 : s0 + na]
                    else:
                        elist = src_pos_sorted[s0 + na : int(estart[nd + 1])] - HALF
                    epos = 0
                    for rho, lane, used in segs:
                        sig[sig_off[k] + soff + rho, lane, col] = 1.0
                        blocks[k][roff[rho] : roff[rho] + used, lane] = elist[
                            epos : epos + used
                        ]
                        epos += used
                    assert epos == len(elist), (c, k, col)
        # flatten idx in group order: per group: A chunks (tile-major), then B
        flat = []
        for g in groups:
            for k in g:
                flat.append(blocksA[k].reshape(-1))
            for k in g:
                flat.append(blocksB[k].reshape(-1))
        flat = np.concatenate(flat)
        assert flat.size == tot_chunks * P
        assert flat.min() >= 0 and flat.max() < HALF
        wrapped = flat.astype(np.int16).reshape(-1, 16).T.copy()
        idx_cores.append(np.tile(wrapped, (8, 1)))
        sigma_cores.append(sig)
        dinv_own_cores.append(dvo)

    return dict(
        T=T,
        T_REAL=T_REAL,
        SLOTS=SLOTS,
        CA=CA,
        CB=CB,
        rangesA=rangesA,
        rangesB=rangesB,
        sig_off=sig_off,
        NSIG=NSIG,
        groups=groups,
        tot_chunks=tot_chunks,
        core_of=core_of,
        slot_of=slot_of,
        pos_of=pos_of,
        dinv=dinv,
        idx_cores=idx_cores,
        sigma_cores=sigma_cores,
        dinv_own_cores=dinv_own_cores,
        xs_perm=xs_perm,
    )


# ----------------------------------------------------------------------------
# Device kernel
# ----------------------------------------------------------------------------
def _build(plan):
    import concourse.bass as bass
    import concourse.mybir as mybir
    import concourse.tile as tile
    from concourse import bacc

    f16 = mybir.dt.float16
    f32 = mybir.dt.float32
    i16 = mybir.dt.int16

    T = plan["T"]
    T_REAL = plan["T_REAL"]
    SLOTS = plan["SLOTS"]
    CA, CB = plan["CA"], plan["CB"]
    rangesA, rangesB = plan["rangesA"], plan["rangesB"]
    sig_off = plan["sig_off"]
    NSIG = plan["NSIG"]
    groups = plan["groups"]
    tot_chunks = plan["tot_chunks"]
    ROWS = NCORES * SLOTS
    HALFROWS = ROWS // 2
    max_ca = max(int(sum(CA[k] for k in g)) for g in groups)
    max_cb = max(int(sum(CB[k] for k in g)) for g in groups)
    max_sig = max(int(sig_off[g[-1] + 1] - sig_off[g[0]]) for g in groups)

    nc = bacc.Bacc(
        "TRN2",
        target_bir_lowering=False,
        num_devices=NCORES,
        num_swdge_queues=SWDGE_QUEUES,
    )
    qn = [0]

    def _next_q():
        qn[0] = (qn[0] + 1) % SWDGE_QUEUES
        return qn[0]

    xs_in = nc.dram_tensor("xs", [ROWS, IN_CH], f16, kind="ExternalInput")
    xso_in = nc.dram_tensor("xs_own", [SLOTS, IN_CH], f16, kind="ExternalInput")
    w1_in = nc.dram_tensor("W1", [IN_CH, HID], f16, kind="ExternalInput")
    w2_in = nc.dram_tensor("W2", [HID, OUT_CH], f16, kind="ExternalInput")
    b1_in = nc.dram_tensor("b1bc", [P, HID], f32, kind="ExternalInput")
    b2_in = nc.dram_tensor("b2bc", [P, OUT_CH], f32, kind="ExternalInput")
    id_in = nc.dram_tensor("ident", [P, P], f16, kind="ExternalInput")
    sig_in = nc.dram_tensor("sigma", [NSIG, P, P], f16, kind="ExternalInput")
    do_in = nc.dram_tensor("dinv_own", [P, T], f32, kind="ExternalInput")
    idx_in = nc.dram_tensor("idx", [P, tot_chunks * 8], i16, kind="ExternalInput")
    out_ext = nc.dram_tensor("out", [SLOTS, OUT_CH], f32, kind="ExternalOutput")

    with tile.TileContext(nc) as tc:
        with (
            tc.tile_pool(name="const", bufs=1) as cpool,
            tc.tile_pool(name="sig", bufs=STG_BUFS) as sigpool,
            tc.tile_pool(name="stg", bufs=STG_BUFS) as stgpool,
            tc.tile_pool(name="own", bufs=STG_BUFS) as ownpool,
            tc.tile_pool(name="drain", bufs=3) as dpool,
            tc.tile_pool(name="wb", bufs=2) as wbpool,
            tc.tile_pool(name="psa", bufs=3, space="PSUM") as ps_agg,
            tc.tile_pool(name="pst", bufs=2, space="PSUM") as ps_tr,
            tc.tile_pool(name="psm", bufs=2, space="PSUM") as ps_mm,
            tc.tile_pool(name="dram", bufs=1, space="DRAM") as dram,
        ):
            # ---- constants into SBUF ----
            w1_sb = cpool.tile([IN_CH, HID], f16)
            nc.sync.dma_start(out=w1_sb[:], in_=w1_in[:])
            w2_sb = cpool.tile([HID, OUT_CH], f16)
            nc.sync.dma_start(out=w2_sb[:], in_=w2_in[:])
            b1_sb = cpool.tile([P, HID], f32)
            nc.sync.dma_start(out=b1_sb[:], in_=b1_in[:])
            b2_sb = cpool.tile([P, OUT_CH], f32)
            nc.sync.dma_start(out=b2_sb[:], in_=b2_in[:])
            id_sb = cpool.tile([P, P], f16)
            nc.sync.dma_start(out=id_sb[:], in_=id_in[:])
            do_sb = cpool.tile([P, T], f32)
            nc.sync.dma_start(out=do_sb[:], in_=do_in[:])
            idx_sb = cpool.tile([P, tot_chunks * 8], i16)
            nc.sync.dma_start(out=idx_sb[:], in_=idx_in[:])
            r3_sb = cpool.tile([P, T_REAL * P], f16)  # layer-2 own rows

            shard2 = dram.tile([SLOTS, HID], f16)
            table2 = dram.tile([ROWS, HID], f16, addr_space="Shared")

            # zero the spare tile of shard2 (PAD rows of table2)
            zt = cpool.tile([P, HID], f16)
            nc.vector.memset(zt[:], 0.0)
            nc.sync.dma_start(out=shard2[T_REAL * P : SLOTS, :], in_=zt[:])

            def drain(layer, k, gi, aps, wb):
                dv = do_sb[:, k : k + 1]
                # agg (PSUM f32) -> f16, transpose, then apply W
                s16 = dpool.tile([P, P], f16, tag="s16")
                nc.scalar.activation(
                    s16[:], aps[:, 0, :], mybir.ActivationFunctionType.Copy
                )
                psT = ps_tr.tile([P, P], f16, tag="tr")
                nc.tensor.transpose(psT[:], s16[:], id_sb[:])
                rT = dpool.tile([P, P], f16, tag="rT")
                nc.scalar.activation(rT[:], psT[:], mybir.ActivationFunctionType.Copy)
                if layer == 0:
                    ps2 = ps_mm.tile([P, HID], f32, tag="mm")
                    nc.tensor.matmul(
                        ps2[:], lhsT=rT[:], rhs=w1_sb[:], start=True, stop=True
                    )
                    r1 = dpool.tile([P, HID], f32, tag="r1")
                    nc.scalar.activation(
                        r1[:], ps2[:], mybir.ActivationFunctionType.Copy, scale=dv
                    )
                    nc.vector.tensor_add(r1[:], r1[:], b1_sb[:])
                    # r3 = dinv*relu(r1) = relu(dinv*r1)  (dinv > 0), f16
                    nc.scalar.activation(
                        r3_sb[:, k * P : (k + 1) * P],
                        r1[:],
                        mybir.ActivationFunctionType.Relu,
                        scale=dv,
                    )
                else:
                    ps2 = ps_mm.tile([P, OUT_CH], f32, tag="mm")
                    nc.tensor.matmul(
                        ps2[:], lhsT=rT[:], rhs=w2_sb[:], start=True, stop=True
                    )
                    o1 = dpool.tile([P, OUT_CH], f32, tag="o1")
                    nc.scalar.activation(
                        o1[:], ps2[:], mybir.ActivationFunctionType.Copy, scale=dv
                    )
                    nc.vector.tensor_add(wb[:, gi, :], o1[:], b2_sb[:])

            def aggregate(layer):
                tab = xs_in if layer == 0 else table2
                coff = 0
                for gidx, g in enumerate(groups):
                    ca_g = int(sum(int(CA[k]) for k in g))
                    cb_g = int(sum(int(CB[k]) for k in g))
                    stA = stB = None
                    if ca_g:
                        stA = stgpool.tile([P, max_ca, P], f16, tag="stgA")
                        for s_ in range(0, ca_g, CALL_CAP):
                            n_ = min(CALL_CAP, ca_g - s_)
                            nc.gpsimd.dma_gather(
                                stA[:, s_ : s_ + n_, :],
                                tab[0:HALFROWS, :],
                                idx_sb[:, (coff + s_) * 8 : (coff + s_ + n_) * 8],
                                n_ * P,
                                n_ * P,
                                P,
                                queue_num=_next_q(),
                            )
                    if cb_g:
                        stB = stgpool.tile([P, max_cb, P], f16, tag="stgB")
                        for s_ in range(0, cb_g, CALL_CAP):
                            n_ = min(CALL_CAP, cb_g - s_)
                            nc.gpsimd.dma_gather(
                                stB[:, s_ : s_ + n_, :],
                                tab[HALFROWS:ROWS, :],
                                idx_sb[
                                    :,
                                    (coff + ca_g + s_) * 8 : (coff + ca_g + s_ + n_)
                                    * 8,
                                ],
                                n_ * P,
                                n_ * P,
                                P,
                                queue_num=_next_q(),
                            )
                    # sigma slab + own rows for this group (batched dmas)
                    s0, s1 = int(sig_off[g[0]]), int(sig_off[g[-1] + 1])
                    sg = sigpool.tile([P, max_sig, P], f16, tag="sig")
                    if s1 > s0:
                        nc.sync.dma_start(
                            out=sg[:, 0 : s1 - s0, :],
                            in_=sig_in[s0:s1].rearrange("s p q -> p s q"),
                        )
                    if layer == 0:
                        own = ownpool.tile([P, GSZ, IN_CH], f16, tag="own")
                        nc.sync.dma_start(
                            out=own[:, 0 : len(g), :],
                            in_=xso_in[g[0] * P : (g[-1] + 1) * P, :].rearrange(
                                "(t p) f -> p t f", p=P
                            ),
                        )
                        wb = None
                    else:
                        wb = wbpool.tile([P, GSZ, OUT_CH], f32, tag="wb1")
                    a_off = 0
                    b_off = 0
                    for gi, k in enumerate(g):
                        aps = ps_agg.tile([P, 1, P], f32, tag="agg")
                        # self-loop rows via identity matmul (starts PSUM)
                        own_rhs = (
                            own[:, gi, :]
                            if layer == 0
                            else r3_sb[:, k * P : (k + 1) * P]
                        )
                        nranges = len(rangesA[k]) + len(rangesB[k])
                        nc.tensor.matmul(
                            aps[:, 0, :],
                            lhsT=id_sb[:],
                            rhs=own_rhs,
                            start=True,
                            stop=(nranges == 0),
                        )
                        nchunks = int(CA[k]) + int(CB[k])
                        done = 0
                        soff = int(sig_off[k]) - s0
                        for rngs, st, off0 in (
                            (rangesA[k], stA, a_off),
                            (rangesB[k], stB, b_off),
                        ):
                            ci = 0
                            for L in rngs:
                                sl = sg[:, soff, :]
                                for _ in range(L):
                                    nc.tensor.matmul(
                                        aps[:, 0, :],
                                        lhsT=sl,
                                        rhs=st[:, off0 + ci, :],
                                        start=False,
                                        stop=(done == nchunks - 1),
                                    )
                                    ci += 1
                                    done += 1
                                soff += 1
                        a_off += int(CA[k])
                        b_off += int(CB[k])
                        drain(layer, k, gi, aps, wb)
                    # group writeback (1 dma)
                    if layer == 0:
                        nc.sync.dma_start(
                            out=shard2[g[0] * P : (g[-1] + 1) * P, :].rearrange(
                                "(t p) f -> p t f", p=P
                            ),
                            in_=r3_sb[:, g[0] * P : (g[-1] + 1) * P].rearrange(
                                "p (t q) -> p t q", q=P
                            ),
                        )
                    else:
                        nc.sync.dma_start(
                            out=out_ext[g[0] * P : (g[-1] + 1) * P, :].rearrange(
                                "(t p) f -> p t f", p=P
                            ),
                            in_=wb[:, 0 : len(g), :],
                        )
                    coff += ca_g + cb_g

            aggregate(0)

            nc.gpsimd.collective_compute(
                "AllGather",
                mybir.AluOpType.bypass,
                replica_groups=[list(range(NCORES))],
                ins=[shard2.opt()],
                outs=[table2.opt()],
            )

            aggregate(1)

    nc.compile()  # bacc passes: library loads, register allocation, DCE
    import concourse.mybir as mybir2

    _split_sync_waits(nc, mybir2, max_waits=1)
    return nc


def _split_sync_waits(nc, mybir, max_waits=1):
    """This walrus build rejects instructions with more than `max_waits` sync
    waits; hoist excess waits onto injected same-engine InstNoOps."""
    n_split = 0
    for fn in nc.m.functions:
        for bb in fn.blocks:
            out = []
            changed = False
            for ins in bb.instructions:
                si = ins.sync_info
                if si is not None and si.on_wait and len(si.on_wait) > max_waits:
                    waits = list(si.on_wait)
                    excess = waits[:-max_waits]
                    for i in range(0, len(excess), max_waits):
                        nop = mybir.InstNoOp(
                            name=nc.get_next_instruction_name(),
                            sync_info=mybir.SyncInfo(
                                on_wait=excess[i : i + max_waits], on_update=[]
                            ),
                            bass_nofuse=True,
                            engine=ins.engine,
                        )
                        out.append(nop)
                        n_split += 1
                    si.on_wait = waits[-max_waits:]
                    ins.sync_info = si
                    changed = True
                out.append(ins)
            if changed:
                bb.instructions = out
    return n_split


# ----------------------------------------------------------------------------
# Entry point
# ----------------------------------------------------------------------------
def kernel(x, edge_index, W1, b1, W2, b2):
    global LAST_RESULTS
    from concourse.bass_utils import run_bass_kernel_spmd

    x = np.asarray(x)
    W1a = np.asarray(W1)
    b1a = np.asarray(b1)
    W2a = np.asarray(W2)
    b2a = np.asarray(b2)

    key = hash(np.asarray(edge_index)[:, :: E // 997].tobytes())
    if key not in _CACHE:
        plan = _plan(edge_index)
        nc = _build(plan)
        _CACHE[key] = (plan, nc)
    plan, nc = _CACHE[key]

    SLOTS = plan["SLOTS"]

    # xs table: dinv-prescaled x rows in slot layout, f16; zeros elsewhere
    xs_node = np.concatenate(
        [plan["dinv"][:, None] * x, np.zeros((1, IN_CH), np.float32)], axis=0
    ).astype(np.float16)
    xs = xs_node[plan["xs_perm"].reshape(-1)]  # [ROWS, IN_CH]

    in_common = {
        "xs": xs,
        "W1": W1a.astype(np.float16),
        "W2": W2a.astype(np.float16),
        "b1bc": np.broadcast_to(b1a.astype(np.float32), (P, HID)).copy(),
        "b2bc": np.broadcast_to(b2a.astype(np.float32), (P, OUT_CH)).copy(),
        "ident": np.eye(P, dtype=np.float16),
    }
    in_maps = []
    for c in range(NCORES):
        m = dict(in_common)
        m["xs_own"] = np.ascontiguousarray(xs[c * SLOTS : (c + 1) * SLOTS])
        m["sigma"] = plan["sigma_cores"][c]
        m["dinv_own"] = plan["dinv_own_cores"][c]
        m["idx"] = plan["idx_cores"][c]
        in_maps.append(m)

    res = run_bass_kernel_spmd(nc, in_maps, core_ids=list(range(NCORES)))
    LAST_RESULTS = res

    out = np.empty((N, OUT_CH), dtype=np.float32)
    core_of = plan["core_of"]
    slot_of = plan["slot_of"]
    for c in range(NCORES):
        sel = core_of == c
        out[sel] = res.results[c]["out"][slot_of[sel]]
    return out


# revision 12
# speedup vs baseline: 1.0474x; 1.0445x over previous
"""GCN 2-layer encoder on 8 TRN2 NeuronCores (Bass/Tile).

Math (PyG GCNConv, symmetric normalization, self-loops, deg from dst):
    out1 = relu(Dh @ A @ Dh @ (x @ W1) + b1),  Dh = diag(deg^-1/2)
    out  = Dh @ A @ Dh @ (out1 @ W2) + b2

Since aggregation is linear, W is applied AFTER aggregation:
    out1[d] = relu(dinv[d] * (agg1[d] @ W1) + b1),  agg1[d] = sum_{s in N(d)+{d}} xs[s]
    xs[s]   = dinv[s] * x[s]                        (prescaled on HOST -> no phase 1!)
    out[d]  = dinv[d] * (agg2[d] @ W2) + b2,        agg2[d] = sum r3[s]
    r3[s]   = dinv[s] * out1[s]

Sharding: nodes -> 8 cores (balanced total degree). Each core aggregates its own
dst nodes via SWDGE dma_gather of 256B rows + multi-hot sigma matmuls in PSUM.
Self-loops never gathered: one identity matmul per tile adds the own-rows tile
(xs_own for layer 1; SBUF-resident r3 for layer 2).

Gather indices are int16 -> the row table is split in A (cores 0-3) / B halves.
Per (tile, side) edges are packed into chunk*lane cells under up to 3 "sigma
ranges" (chunk intervals sharing one sigma), which kills the old per-node
lane-capacity padding. 4 SWDGE queues + 3 staging groups in flight parallelize
descriptor generation (measured 3.3x vs serial issue).

Layer-2 rows (r3) are exchanged with one AllGather between the layers.
"""

import sys
import types

sys.path.insert(0, "/opt/trn_rl_repo")

import numpy as np

# Register the NTFF profile hook the container's antenv stub lacks, so
# BASS_TRACE=1 profiling works under axon (harmless otherwise).
if "antenv.axon_hooks" not in sys.modules:
    try:
        from trn_agent_boot.trn_boot import _ntff_profile_via_ctypes

        _hook = _ntff_profile_via_ctypes("/opt/axon/libaxon_pjrt.so")
    except Exception:
        _hook = None
    _m = types.ModuleType("antenv.axon_hooks")
    _m.get_axon_ntff_profile_hook = lambda: _hook
    sys.modules["antenv.axon_hooks"] = _m

N = 50000
E = 800000
IN_CH = 128
HID = 128
OUT_CH = 64
NCORES = 8
P = 128
GSZ = 4  # tiles per gather/stage group
CALL_CAP = 8  # max chunks (x128 idxs) per dma_gather call; larger crashes HW
SWDGE_QUEUES = 4
STG_BUFS = 6  # staging groups in flight
AG_LAG = 2  # groups between a shard2 write and its partial AllGather dispatch

_CACHE = {}
LAST_RESULTS = None


# ----------------------------------------------------------------------------
# Host-side planning
# ----------------------------------------------------------------------------
def _range_split(C):
    """Split C chunks into <=3 sigma-range lengths."""
    l1 = (C + 1) // 2
    l2 = (C - l1 + 1) // 2
    l3 = C - l1 - l2
    return [l for l in (l1, l2, l3) if l > 0]


def _pack_side(counts, C):
    """Pack nodes with cell-counts `counts` (list of (col, c)) into 128 lanes
    x C chunks under sigma ranges _range_split(C).

    A lane column is split into one segment per range; each segment belongs to
    one node. Full lanes (all ranges, same node) are allocated from lane 0 up;
    partial segments from lane 127 down, per range independently.

    Returns None if infeasible, else (range_lengths, [(col, [(rho, lane,
    n_used), ...]), ...])."""
    L = _range_split(C)
    R = len(L)
    nf = 0  # full lanes used (lanes 0..nf-1)
    u = [0] * R  # partial segments used per range (lanes 127-u[rho]+1..127)
    out = []
    for col, c in sorted(counts, key=lambda t: -t[1]):
        segs = []
        f = c // C
        rem = c - f * C
        if nf + f > 128 or any(nf + f + uu > 128 for uu in u):
            return None
        for i in range(f):
            lane = nf + i
            for rho in range(R):
                segs.append((rho, lane, L[rho]))
        nf += f
        # remainder: greedy largest L[rho] <= rem, else smallest available
        while rem > 0:
            best = -1
            for rho in range(R):
                if nf + u[rho] >= 128:
                    continue
                if L[rho] <= rem and (best < 0 or L[rho] > L[best]):
                    best = rho
            if best < 0:
                cand = [rho for rho in range(R) if nf + u[rho] < 128]
                if not cand:
                    return None
                best = min(cand, key=lambda rho: L[rho])
            lane = 127 - u[best]
            used = min(rem, L[best])
            segs.append((best, lane, used))
            u[best] += 1
            rem -= used
        out.append((col, segs))
    return L, out


def _min_C(counts):
    tot = sum(c for _, c in counts)
    C = max(1, -(-tot // 128))
    while True:
        if _pack_side(counts, C) is not None:
            return C
        C += 1


def _plan(edge_index):
    src = np.asarray(edge_index[0], dtype=np.int64)
    dst = np.asarray(edge_index[1], dtype=np.int64)
    deg = np.bincount(dst, minlength=N) + 1  # + self-loop
    dinv = (1.0 / np.sqrt(deg.astype(np.float64))).astype(np.float32)

    # --- node -> core (snake over degree-sorted nodes: balances sum(deg)) ---
    order = np.argsort(-deg, kind="stable")
    snake = np.tile(
        np.concatenate([np.arange(NCORES), np.arange(NCORES - 1, -1, -1)]),
        N // (2 * NCORES) + 1,
    )[:N]
    core_of = np.empty(N, dtype=np.int64)
    core_of[order] = snake

    T_REAL = -(-(N // NCORES) // P)  # 49
    T = T_REAL + 1  # +1 empty tile = guaranteed zero rows for PAD index
    SLOTS = T * P
    HALF = (NCORES // 2) * SLOTS
    assert HALF <= 32768, (T, SLOTS)

    # --- per-dst A/B in-edge counts (A = src on cores 0-3), no self-loops ---
    isA = core_of[src] < (NCORES // 2)
    a_cnt = np.bincount(dst[isA], minlength=N)
    b_cnt = np.bincount(dst[~isA], minlength=N)

    # --- nodes -> tiles (LPT greedy on a+b, <=128 per tile) -----------------
    core_tiles = []  # [core] -> list of T_REAL node-id lists
    for c in range(NCORES):
        nodes = np.where(core_of == c)[0]
        w = a_cnt[nodes] + b_cnt[nodes]
        o2 = np.argsort(-w, kind="stable")
        loads = np.zeros(T_REAL, dtype=np.int64)
        fills = np.zeros(T_REAL, dtype=np.int64)
        tiles = [[] for _ in range(T_REAL)]
        for i in o2:
            open_ = np.where(fills < P)[0]
            t = open_[np.argmin(loads[open_])]
            tiles[t].append(nodes[i])
            loads[t] += w[i]
            fills[t] += 1
        o3 = np.argsort(-loads, kind="stable")  # sort tiles desc by load
        core_tiles.append([tiles[t] for t in o3])

    # --- per (core, tile, side) minimal chunk counts ------------------------
    minC = np.zeros((NCORES, T_REAL, 2), dtype=np.int64)
    counts_all = {}
    for c in range(NCORES):
        for k in range(T_REAL):
            nds = core_tiles[c][k]
            ca = [(j, int(a_cnt[nd])) for j, nd in enumerate(nds) if a_cnt[nd] > 0]
            cb = [(j, int(b_cnt[nd])) for j, nd in enumerate(nds) if b_cnt[nd] > 0]
            counts_all[(c, k)] = (ca, cb)
            minC[c, k, 0] = _min_C(ca) if ca else 0
            minC[c, k, 1] = _min_C(cb) if cb else 0
    CA = minC[:, :, 0].max(axis=0)  # shared per-tile chunk counts (SPMD)
    CB = minC[:, :, 1].max(axis=0)

    # --- slot/position assignment ------------------------------------------
    slot_of = np.full(N, -1, dtype=np.int64)
    for c in range(NCORES):
        for k in range(T_REAL):
            for j, nd in enumerate(core_tiles[c][k]):
                slot_of[nd] = k * P + j
    pos_of = core_of * SLOTS + slot_of

    # --- edge lists grouped by (dst, side), sources as table positions ------
    sideval = (~isA).astype(np.int64)
    eorder = np.argsort(dst * 2 + sideval, kind="stable")
    src_pos_sorted = pos_of[src[eorder]]
    estart = np.zeros(N + 1, dtype=np.int64)
    np.cumsum(a_cnt + b_cnt, out=estart[1:])

    # --- groups -------------------------------------------------------------
    G = -(-T_REAL // GSZ)
    groups = [list(range(g * GSZ, min((g + 1) * GSZ, T_REAL))) for g in range(G)]
    PAD = SLOTS - 1  # empty-tile row of core 0 (A half) / core 4 (B half)

    # --- sigma layout: per tile [A ranges..., B ranges...], stacked ---------
    tot_chunks = int(CA.sum() + CB.sum())
    rangesA = [_range_split(int(C)) if C else [] for C in CA]
    rangesB = [_range_split(int(C)) if C else [] for C in CB]
    nsig_tile = [len(rangesA[k]) + len(rangesB[k]) for k in range(T_REAL)]
    sig_off = np.zeros(T_REAL + 1, dtype=np.int64)
    np.cumsum(nsig_tile, out=sig_off[1:])
    NSIG = int(sig_off[-1])

    idx_cores = []
    sigma_cores = []
    dinv_own_cores = []
    xs_perm = np.full((NCORES, SLOTS), N, dtype=np.int64)  # node per slot (N=zero)
    for c in range(NCORES):
        blocksA = [np.full((int(CA[k]), P), PAD, np.int64) for k in range(T_REAL)]
        blocksB = [np.full((int(CB[k]), P), PAD, np.int64) for k in range(T_REAL)]
        sig = np.zeros((NSIG, P, P), dtype=np.float16)
        dvo = np.zeros((P, T), dtype=np.float32)
        for k in range(T_REAL):
            nds = core_tiles[c][k]
            for j, nd in enumerate(nds):
                dvo[j, k] = dinv[nd]
                xs_perm[c, k * P + j] = nd
            ca, cb = counts_all[(c, k)]
            for counts, C, rngs, blocks, soff in (
                (ca, int(CA[k]), rangesA[k], blocksA, 0),
                (cb, int(CB[k]), rangesB[k], blocksB, len(rangesA[k])),
            ):
                if not counts:
                    continue
                packed = _pack_side(counts, C)
                assert packed is not None, (c, k, C)
                L, placements = packed
                roff = np.zeros(len(L) + 1, dtype=np.int64)
                np.cumsum(L, out=roff[1:])
                for col, segs in placements:
                    nd = nds[col]
                    s0 = int(estart[nd])
                    na = int(a_cnt[nd])
                    if soff == 0:  # A side
                        elist = src_pos_sorted[s0 : s0 + na]
                    else:
                        elist = src_pos_sorted[s0 + na : int(estart[nd + 1])] - HALF
                    epos = 0
                    for rho, lane, used in segs:
                        sig[sig_off[k] + soff + rho, lane, col] = 1.0
                        blocks[k][roff[rho] : roff[rho] + used, lane] = elist[
                            epos : epos + used
                        ]
                        epos += used
                    assert epos == len(elist), (c, k, col)
        # flatten idx in group order: per group: A chunks (tile-major), then B
        flat = []
        for g in groups:
            for k in g:
                flat.append(blocksA[k].reshape(-1))
            for k in g:
                flat.append(blocksB[k].reshape(-1))
        flat = np.concatenate(flat)
        assert flat.size == tot_chunks * P
        assert flat.min() >= 0 and flat.max() < HALF
        wrapped = flat.astype(np.int16).reshape(-1, 16).T.copy()
        idx_cores.append(np.tile(wrapped, (8, 1)))
        sigma_cores.append(sig)
        dinv_own_cores.append(dvo)

    return dict(
        T=T,
        T_REAL=T_REAL,
        SLOTS=SLOTS,
        CA=CA,
        CB=CB,
        rangesA=rangesA,
        rangesB=rangesB,
        sig_off=sig_off,
        NSIG=NSIG,
        groups=groups,
        tot_chunks=tot_chunks,
        core_of=core_of,
        slot_of=slot_of,
        pos_of=pos_of,
        dinv=dinv,
        idx_cores=idx_cores,
        sigma_cores=sigma_cores,
        dinv_own_cores=dinv_own_cores,
        xs_perm=xs_perm,
    )


# ----------------------------------------------------------------------------
# Device kernel
# ----------------------------------------------------------------------------
def _build(plan):
    import concourse.bass as bass
    import concourse.mybir as mybir
    import concourse.tile as tile
    from concourse import bacc

    f16 = mybir.dt.float16
    f32 = mybir.dt.float32
    i16 = mybir.dt.int16

    T = plan["T"]
    T_REAL = plan["T_REAL"]
    SLOTS = plan["SLOTS"]
    CA, CB = plan["CA"], plan["CB"]
    rangesA, rangesB = plan["rangesA"], plan["rangesB"]
    sig_off = plan["sig_off"]
    NSIG = plan["NSIG"]
    groups = plan["groups"]
    tot_chunks = plan["tot_chunks"]
    ROWS = NCORES * SLOTS
    HALFROWS = ROWS // 2
    max_ca = max(int(sum(CA[k] for k in g)) for g in groups)
    max_cb = max(int(sum(CB[k] for k in g)) for g in groups)
    max_sig = max(int(sig_off[g[-1] + 1] - sig_off[g[0]]) for g in groups)

    nc = bacc.Bacc(
        "TRN2",
        target_bir_lowering=False,
        num_devices=NCORES,
        num_swdge_queues=SWDGE_QUEUES,
    )
    qn = [0]

    def _next_q():
        qn[0] = (qn[0] + 1) % SWDGE_QUEUES
        return qn[0]

    xs_in = nc.dram_tensor("xs", [ROWS, IN_CH], f16, kind="ExternalInput")
    xso_in = nc.dram_tensor("xs_own", [SLOTS, IN_CH], f16, kind="ExternalInput")
    w1_in = nc.dram_tensor("W1", [IN_CH, HID], f16, kind="ExternalInput")
    w2_in = nc.dram_tensor("W2", [HID, OUT_CH], f16, kind="ExternalInput")
    b1_in = nc.dram_tensor("b1bc", [P, HID], f32, kind="ExternalInput")
    b2_in = nc.dram_tensor("b2bc", [P, OUT_CH], f32, kind="ExternalInput")
    id_in = nc.dram_tensor("ident", [P, P], f16, kind="ExternalInput")
    sig_in = nc.dram_tensor("sigma", [NSIG, P, P], f16, kind="ExternalInput")
    do_in = nc.dram_tensor("dinv_own", [P, T], f32, kind="ExternalInput")
    idx_in = nc.dram_tensor("idx", [P, tot_chunks * 8], i16, kind="ExternalInput")
    out_ext = nc.dram_tensor("out", [SLOTS, OUT_CH], f32, kind="ExternalOutput")

    with tile.TileContext(nc) as tc:
        with (
            tc.tile_pool(name="const", bufs=1) as cpool,
            tc.tile_pool(name="sig", bufs=STG_BUFS) as sigpool,
            tc.tile_pool(name="stg", bufs=STG_BUFS) as stgpool,
            tc.tile_pool(name="own", bufs=STG_BUFS) as ownpool,
            tc.tile_pool(name="drain", bufs=3) as dpool,
            tc.tile_pool(name="wb", bufs=2) as wbpool,
            tc.tile_pool(name="psa", bufs=4, space="PSUM") as ps_agg,
            tc.tile_pool(name="pst", bufs=2, space="PSUM") as ps_tr,
            tc.tile_pool(name="psm", bufs=2, space="PSUM") as ps_mm,
            tc.tile_pool(name="dram", bufs=1, space="DRAM") as dram,
        ):
            # ---- constants into SBUF ----
            w1_sb = cpool.tile([IN_CH, HID], f16)
            nc.sync.dma_start(out=w1_sb[:], in_=w1_in[:])
            w2_sb = cpool.tile([HID, OUT_CH], f16)
            nc.sync.dma_start(out=w2_sb[:], in_=w2_in[:])
            b1_sb = cpool.tile([P, HID], f32)
            nc.sync.dma_start(out=b1_sb[:], in_=b1_in[:])
            b2_sb = cpool.tile([P, OUT_CH], f32)
            nc.sync.dma_start(out=b2_sb[:], in_=b2_in[:])
            id_sb = cpool.tile([P, P], f16)
            nc.sync.dma_start(out=id_sb[:], in_=id_in[:])
            do_sb = cpool.tile([P, T], f32)
            nc.sync.dma_start(out=do_sb[:], in_=do_in[:])
            idx_sb = cpool.tile([P, tot_chunks * 8], i16)
            nc.sync.dma_start(out=idx_sb[:], in_=idx_in[:])
            r3_sb = cpool.tile([P, T_REAL * P], f16)  # layer-2 own rows

            shard2 = dram.tile([SLOTS, HID], f16)
            table2 = dram.tile([ROWS, HID], f16, addr_space="Shared")

            # zero the spare tile of shard2 (PAD rows of table2)
            zt = cpool.tile([P, HID], f16)
            nc.vector.memset(zt[:], 0.0)
            nc.sync.dma_start(out=shard2[T_REAL * P : SLOTS, :], in_=zt[:])

            def drain(layer, k, gi, aps, wb):
                dv = do_sb[:, k : k + 1]
                # agg (PSUM f32) -> f16, transpose, then apply W
                s16 = dpool.tile([P, P], f16, tag="s16")
                nc.scalar.activation(
                    s16[:], aps[:, 0, :], mybir.ActivationFunctionType.Copy
                )
                psT = ps_tr.tile([P, P], f16, tag="tr")
                nc.tensor.transpose(psT[:], s16[:], id_sb[:])
                rT = dpool.tile([P, P], f16, tag="rT")
                nc.scalar.activation(rT[:], psT[:], mybir.ActivationFunctionType.Copy)
                if layer == 0:
                    ps2 = ps_mm.tile([P, HID], f32, tag="mm")
                    nc.tensor.matmul(
                        ps2[:], lhsT=rT[:], rhs=w1_sb[:], start=True, stop=True
                    )
                    r1 = dpool.tile([P, HID], f32, tag="r1")
                    nc.scalar.activation(
                        r1[:], ps2[:], mybir.ActivationFunctionType.Copy, scale=dv
                    )
                    nc.vector.tensor_add(r1[:], r1[:], b1_sb[:])
                    # r3 = dinv*relu(r1) = relu(dinv*r1)  (dinv > 0), f16
                    nc.scalar.activation(
                        r3_sb[:, k * P : (k + 1) * P],
                        r1[:],
                        mybir.ActivationFunctionType.Relu,
                        scale=dv,
                    )
                else:
                    ps2 = ps_mm.tile([P, OUT_CH], f32, tag="mm")
                    nc.tensor.matmul(
                        ps2[:], lhsT=rT[:], rhs=w2_sb[:], start=True, stop=True
                    )
                    o1 = dpool.tile([P, OUT_CH], f32, tag="o1")
                    nc.scalar.activation(
                        o1[:], ps2[:], mybir.ActivationFunctionType.Copy, scale=dv
                    )
                    nc.vector.tensor_add(wb[:, gi, :], o1[:], b2_sb[:])

            def aggregate(layer):
                tab = xs_in if layer == 0 else table2
                coff = 0
                for gidx, g in enumerate(groups):
                    ca_g = int(sum(int(CA[k]) for k in g))
                    cb_g = int(sum(int(CB[k]) for k in g))
                    stA = stB = None
                    if ca_g:
                        stA = stgpool.tile([P, max_ca, P], f16, tag="stgA")
                        for s_ in range(0, ca_g, CALL_CAP):
                            n_ = min(CALL_CAP, ca_g - s_)
                            nc.gpsimd.dma_gather(
                                stA[:, s_ : s_ + n_, :],
                                tab[0:HALFROWS, :],
                                idx_sb[:, (coff + s_) * 8 : (coff + s_ + n_) * 8],
                                n_ * P,
                                n_ * P,
                                P,
                                queue_num=_next_q(),
                            )
                    if cb_g:
                        stB = stgpool.tile([P, max_cb, P], f16, tag="stgB")
                        for s_ in range(0, cb_g, CALL_CAP):
                            n_ = min(CALL_CAP, cb_g - s_)
                            nc.gpsimd.dma_gather(
                                stB[:, s_ : s_ + n_, :],
                                tab[HALFROWS:ROWS, :],
                                idx_sb[
                                    :,
                                    (coff + ca_g + s_) * 8 : (coff + ca_g + s_ + n_)
                                    * 8,
                                ],
                                n_ * P,
                                n_ * P,
                                P,
                                queue_num=_next_q(),
                            )
                    # sigma slab + own rows for this group (batched dmas)
                    s0, s1 = int(sig_off[g[0]]), int(sig_off[g[-1] + 1])
                    sg = sigpool.tile([P, max_sig, P], f16, tag="sig")
                    if s1 > s0:
                        nc.sync.dma_start(
                            out=sg[:, 0 : s1 - s0, :],
                            in_=sig_in[s0:s1].rearrange("s p q -> p s q"),
                        )
                    if layer == 0:
                        own = ownpool.tile([P, GSZ, IN_CH], f16, tag="own")
                        nc.sync.dma_start(
                            out=own[:, 0 : len(g), :],
                            in_=xso_in[g[0] * P : (g[-1] + 1) * P, :].rearrange(
                                "(t p) f -> p t f", p=P
                            ),
                        )
                        wb = None
                    else:
                        wb = wbpool.tile([P, GSZ, OUT_CH], f32, tag="wb1")
                    a_off = 0
                    b_off = 0
                    for gi, k in enumerate(g):
                        aps = ps_agg.tile([P, 1, P], f32, tag="agg")
                        # self-loop rows via identity matmul (starts PSUM)
                        own_rhs = (
                            own[:, gi, :]
                            if layer == 0
                            else r3_sb[:, k * P : (k + 1) * P]
                        )
                        nranges = len(rangesA[k]) + len(rangesB[k])
                        nc.tensor.matmul(
                            aps[:, 0, :],
                            lhsT=id_sb[:],
                            rhs=own_rhs,
                            start=True,
                            stop=(nranges == 0),
                        )
                        nchunks = int(CA[k]) + int(CB[k])
                        done = 0
                        soff = int(sig_off[k]) - s0
                        for rngs, st, off0 in (
                            (rangesA[k], stA, a_off),
                            (rangesB[k], stB, b_off),
                        ):
                            ci = 0
                            for L in rngs:
                                sl = sg[:, soff, :]
                                for _ in range(L):
                                    nc.tensor.matmul(
                                        aps[:, 0, :],
                                        lhsT=sl,
                                        rhs=st[:, off0 + ci, :],
                                        start=False,
                                        stop=(done == nchunks - 1),
                                    )
                                    ci += 1
                                    done += 1
                                soff += 1
                        a_off += int(CA[k])
                        b_off += int(CB[k])
                        drain(layer, k, gi, aps, wb)
                    # group writeback (1 dma)
                    if layer == 0:
                        nc.sync.dma_start(
                            out=shard2[g[0] * P : (g[-1] + 1) * P, :].rearrange(
                                "(t p) f -> p t f", p=P
                            ),
                            in_=r3_sb[:, g[0] * P : (g[-1] + 1) * P].rearrange(
                                "p (t q) -> p t q", q=P
                            ),
                        )
                    else:
                        nc.sync.dma_start(
                            out=out_ext[g[0] * P : (g[-1] + 1) * P, :].rearrange(
                                "(t p) f -> p t f", p=P
                            ),
                            in_=wb[:, 0 : len(g), :],
                        )
                    coff += ca_g + cb_g

            aggregate(0)

            nc.gpsimd.collective_compute(
                "AllGather",
                mybir.AluOpType.bypass,
                replica_groups=[list(range(NCORES))],
                ins=[shard2.opt()],
                outs=[table2.opt()],
            )

            aggregate(1)

    nc.compile()  # bacc passes: library loads, register allocation, DCE
    import concourse.mybir as mybir2

    _split_sync_waits(nc, mybir2, max_waits=1)
    return nc


def _split_sync_waits(nc, mybir, max_waits=1):
    """This walrus build rejects instructions with more than `max_waits` sync
    waits; hoist excess waits onto injected same-engine InstNoOps."""
    n_split = 0
    for fn in nc.m.functions:
        for bb in fn.blocks:
            out = []
            changed = False
            for ins in bb.instructions:
                si = ins.sync_info
                if si is not None and si.on_wait and len(si.on_wait) > max_waits:
                    waits = list(si.on_wait)
                    excess = waits[:-max_waits]
                    for i in range(0, len(excess), max_waits):
                        nop = mybir.InstNoOp(
                            name=nc.get_next_instruction_name(),
                            sync_info=mybir.SyncInfo(
                                on_wait=excess[i : i + max_waits], on_update=[]
                            ),
                            bass_nofuse=True,
                            engine=ins.engine,
                        )
                        out.append(nop)
                        n_split += 1
                    si.on_wait = waits[-max_waits:]
                    ins.sync_info = si
                    changed = True
                out.append(ins)
            if changed:
                bb.instructions = out
    return n_split


# ----------------------------------------------------------------------------
# Entry point
# ----------------------------------------------------------------------------
def kernel(x, edge_index, W1, b1, W2, b2):
    global LAST_RESULTS
    from concourse.bass_utils import run_bass_kernel_spmd

    x = np.asarray(x)
    W1a = np.asarray(W1)
    b1a = np.asarray(b1)
    W2a = np.asarray(W2)
    b2a = np.asarray(b2)

    key = hash(np.asarray(edge_index)[:, :: E // 997].tobytes())
    if key not in _CACHE:
        plan = _plan(edge_index)
        nc = _build(plan)
        _CACHE[key] = (plan, nc)
    plan, nc = _CACHE[key]

    SLOTS = plan["SLOTS"]

    # xs table: dinv-prescaled x rows in slot layout, f16; zeros elsewhere
    xs_node = np.concatenate(
        [plan["dinv"][:, None] * x, np.zeros((1, IN_CH), np.float32)], axis=0
    ).astype(np.float16)
    xs = xs_node[plan["xs_perm"].reshape(-1)]  # [ROWS, IN_CH]

    in_common = {
        "xs": xs,
        "W1": W1a.astype(np.float16),
        "W2": W2a.astype(np.float16),
        "b1bc": np.broadcast_to(b1a.astype(np.float32), (P, HID)).copy(),
        "b2bc": np.broadcast_to(b2a.astype(np.float32), (P, OUT_CH)).copy(),
        "ident": np.eye(P, dtype=np.float16),
    }
    in_maps = []
    for c in range(NCORES):
        m = dict(in_common)
        m["xs_own"] = np.ascontiguousarray(xs[c * SLOTS : (c + 1) * SLOTS])
        m["sigma"] = plan["sigma_cores"][c]
        m["dinv_own"] = plan["dinv_own_cores"][c]
        m["idx"] = plan["idx_cores"][c]
        in_maps.append(m)

    res = run_bass_kernel_spmd(nc, in_maps, core_ids=list(range(NCORES)))
    LAST_RESULTS = res

    out = np.empty((N, OUT_CH), dtype=np.float32)
    core_of = plan["core_of"]
    slot_of = plan["slot_of"]
    for c in range(NCORES):
        sel = core_of == c
        out[sel] = res.results[c]["out"][slot_of[sel]]
    return out
